# revision 8
# baseline (speedup 1.0000x reference)
"""Trainium2 Bass kernel for nn_DiffNet (2-layer LSTM encoder/decoder + FC head).

Sharding: tensor-parallel over the hidden/gate dimension across 8 NeuronCores.
Core k owns hidden rows [k*128, (k+1)*128) of both LSTM layers plus the
matching rows of fc_w1 / columns of fc_w2.  Activations are [hidden, batch] so
the full batch (256) is the matmul moving dimension.

v3 pipeline:
- Two AllGathers per step (h0, h1), each issued the moment its payload is
  ready; the other layer's matmuls run during the in-flight collective.
- Emission order keeps both recurrence loops at period ~= AG-chain + tail:
  encode: [x_s | hh0_s -> AG0_s | wih1(L1_{s-1}) | whh1(L1_{s-1}) -> AG1].
  whh1 is the LAST accumulation of L1 so its operand (the h1 gather) can land
  one full period late.
- Gathers land in two k-halves; path-critical contractions emit all gates'
  first-half matmuls before any second-half ones, so half the tail hides
  under the gather DMA.
- Decode folds fc2 into the layer-0 gate weights (Wc = W_est @ fc_w2), so
  the gates close straight off the tanh layer; the est recursion
  (est_t = est_{t-1} + fc_w2 u_t + fc_b2) runs off the critical path.
  W_est@fc_b2 is folded into the layer-0 bias on the host.

Self-contained: hardcodes all shapes; host-side numpy only reshapes/slices.
"""

import os

import numpy as np

L = 2
H = 1024
XD = 192
YD = 64
IN = XD + YD  # 256
B = 256
PRE_LEN = int(os.environ.get("DIFFNET_PRE", "64"))
FWD_LEN = int(os.environ.get("DIFFNET_FWD", "48"))
NCORES = 8
SL = H // NCORES  # 128 hidden rows per core
KT_H = H // 128  # 8 K-tiles to contract over a full hidden vector
KH = KT_H // 2  # k-tiles per gather half
NGATE = 4

_CACHE = {}


def _shard_host(inputs):
    """Build per-core input dicts (numpy only: slice / transpose / reshape)."""
    f32 = np.float32

    pre_x = np.asarray(inputs["pre_x"], f32)
    pre_y = np.asarray(inputs["pre_y"], f32)
    fwd_x = np.asarray(inputs["forward_x"], f32)

    # Encoder input, step-major, [t, p(128), kt(2), b] so the DMA is contiguous
    xy = np.concatenate([pre_x, pre_y], axis=2)  # (PRE, B, IN)
    xpre = (
        xy.transpose(0, 2, 1)  # (PRE, IN, B)
        .reshape(PRE_LEN, 2, 128, B)
        .transpose(0, 2, 1, 3)  # (PRE, 128, 2, B)
        .astype(np.float16)
    )
    # Decoder exogenous input: [t, in(192), b]
    xfwd = fwd_x.transpose(0, 2, 1).astype(np.float16)  # (FWD, 192, B)

    w_ih_0 = np.asarray(inputs["w_ih_0"], f32).reshape(NGATE, H, IN)
    w_hh_0 = np.asarray(inputs["w_hh_0"], f32).reshape(NGATE, H, H)
    w_ih_1 = np.asarray(inputs["w_ih_1"], f32).reshape(NGATE, H, H)
    w_hh_1 = np.asarray(inputs["w_hh_1"], f32).reshape(NGATE, H, H)
    b0 = (np.asarray(inputs["b_ih_0"], f32) + np.asarray(inputs["b_hh_0"], f32)).reshape(NGATE, H)
    b1 = (np.asarray(inputs["b_ih_1"], f32) + np.asarray(inputs["b_hh_1"], f32)).reshape(NGATE, H)
    fc_w1 = np.asarray(inputs["fc_w1"], f32)
    fc_b1 = np.asarray(inputs["fc_b1"], f32)
    fc_w2 = np.asarray(inputs["fc_w2"], f32)
    fc_b2 = np.asarray(inputs["fc_b2"], f32)

    def lhsT_tiles(sl_w):
        """(4, 128, K) gate-major rows-for-this-core -> lhsT [128, KT*4*128]."""
        kdim = sl_w.shape[2]
        kt = kdim // 128
        return (
            sl_w.transpose(2, 0, 1)  # (K, 4, 128)
            .reshape(kt, 128, NGATE, SL)
            .transpose(1, 0, 2, 3)  # (128, kt, 4, 128)
            .reshape(128, kt * NGATE * SL)
            .copy()
        )

    maps = []
    for k in range(NCORES):
        sl = slice(k * SL, (k + 1) * SL)
        w0xT = lhsT_tiles(w_ih_0[:, sl, :])  # (128, 2*4*128)
        west = w_ih_0[:, sl, XD:]  # (4, 128, 64)
        westT = west.transpose(2, 0, 1).reshape(YD, NGATE * SL).copy()  # (64, 512)
        whh0T = lhsT_tiles(w_hh_0[:, sl, :])  # (128, 8*4*128)
        wih1T = lhsT_tiles(w_ih_1[:, sl, :])
        whh1T = lhsT_tiles(w_hh_1[:, sl, :])
        # fc2 folded into the layer-0 gates: Wc[g,r,:] = west[g,r,:] @ fc_w2
        wc = np.einsum("grc,ch->grh", west.astype(np.float64),
                       fc_w2.astype(np.float64)).astype(f32)  # (4, 128, H)
        wcT = lhsT_tiles(wc)  # (128, 8*4*128)
        # ... and W_est @ fc_b2 into the layer-0 bias (decode only!)
        b0d = b0[:, sl] + west.astype(np.float64) @ fc_b2.astype(np.float64)  # (4,128)
        # FC head replicated on every core (small): no collective for est.
        fcw1T = (
            fc_w1.T.reshape(KT_H, 128, H).transpose(1, 0, 2).reshape(128, KT_H * H).copy()
        )
        fcw2T = (
            fc_w2.T.reshape(KT_H, 128, YD).transpose(1, 0, 2).reshape(128, KT_H * YD).copy()
        )
        m = {
            "xpre": xpre,
            "xfwd": xfwd,
            "w0xT": w0xT.astype(np.float16),
            "westT": westT.astype(np.float16),
            "whh0T": whh0T.astype(np.float16),
            "wih1T": wih1T.astype(np.float16),
            "whh1T": whh1T.astype(np.float16),
            "wcT": wcT.astype(np.float16),
            "fcw1T": fcw1T.astype(np.float16),
            "fcw2T": fcw2T.astype(np.float16),
            "b0": b0[:, sl].T.copy(),  # (128, 4)
            "b0d": np.asarray(b0d, f32).T.copy(),  # (128, 4) decode bias
            "b1": b1[:, sl].T.copy(),
            "fcb1": fc_b1.reshape(KT_H, 128).T.copy(),  # (128, 8): bias per M-tile
            "fcb2": fc_b2.reshape(YD, 1).copy(),
            "lastyT": pre_y[-1].T.copy(),  # (64, 256)
        }
        maps.append(m)
    return maps


def _build_program():
    import concourse.bass as bass
    import concourse.mybir as mybir
    import concourse.tile as tile
    from concourse import bacc

    dt = mybir.dt
    AF = mybir.ActivationFunctionType
    F32 = dt.float32
    FR = dt.float16  # matmul operand dtype (FWL stays on, ~8x bf16 precision)

    nc = bacc.Bacc("TRN2", target_bir_lowering=False, debug=False, num_devices=NCORES)

    # ---- external I/O ----
    t_xpre = nc.dram_tensor("xpre", [PRE_LEN, 128, 2, B], FR, kind="ExternalInput")
    t_xfwd = nc.dram_tensor("xfwd", [FWD_LEN, XD, B], FR, kind="ExternalInput")
    t_w0xT = nc.dram_tensor("w0xT", [128, 2 * NGATE * SL], FR, kind="ExternalInput")
    t_westT = nc.dram_tensor("westT", [YD, NGATE * SL], FR, kind="ExternalInput")
    t_whh0T = nc.dram_tensor("whh0T", [128, KT_H * NGATE * SL], FR, kind="ExternalInput")
    t_wih1T = nc.dram_tensor("wih1T", [128, KT_H * NGATE * SL], FR, kind="ExternalInput")
    t_whh1T = nc.dram_tensor("whh1T", [128, KT_H * NGATE * SL], FR, kind="ExternalInput")
    t_wcT = nc.dram_tensor("wcT", [128, KT_H * NGATE * SL], FR, kind="ExternalInput")
    t_fcw1T = nc.dram_tensor("fcw1T", [128, KT_H * H], FR, kind="ExternalInput")
    t_fcw2T = nc.dram_tensor("fcw2T", [128, KT_H * YD], FR, kind="ExternalInput")
    t_b0 = nc.dram_tensor("b0", [128, NGATE], F32, kind="ExternalInput")
    t_b0d = nc.dram_tensor("b0d", [128, NGATE], F32, kind="ExternalInput")
    t_b1 = nc.dram_tensor("b1", [128, NGATE], F32, kind="ExternalInput")
    t_fcb1 = nc.dram_tensor("fcb1", [128, KT_H], F32, kind="ExternalInput")
    t_fcb2 = nc.dram_tensor("fcb2", [YD, 1], F32, kind="ExternalInput")
    t_lastyT = nc.dram_tensor("lastyT", [YD, B], F32, kind="ExternalInput")
    t_out = nc.dram_tensor("est_out", [FWD_LEN, YD, B], F32, kind="ExternalOutput")

    RG = [list(range(NCORES))]

    with tile.TileContext(nc) as tc:
        with (
            tc.tile_pool(name="const", bufs=1) as const,
            tc.tile_pool(name="xload", bufs=3) as xload,
            tc.tile_pool(name="state", bufs=4) as state,
            tc.tile_pool(name="gact", bufs=6) as gact,
            tc.tile_pool(name="hfull", bufs=3) as hfull,
            tc.tile_pool(name="psum", bufs=8, space="PSUM") as psum,
            tc.tile_pool(name="dbounce", bufs=8, space="DRAM") as dbounce,
            tc.tile_pool(name="dshared", bufs=8, space="DRAM") as dshared,
        ):
            # ---- load constants ----
            w0xT = const.tile([128, 2, NGATE, SL], FR)
            nc.sync.dma_start(out=w0xT, in_=t_w0xT.ap().rearrange("p (k g m) -> p k g m", k=2, g=NGATE))
            westT = const.tile([YD, NGATE, SL], FR)
            nc.sync.dma_start(out=westT, in_=t_westT.ap().rearrange("p (g m) -> p g m", g=NGATE))
            whh0T = const.tile([128, KT_H, NGATE, SL], FR)
            nc.sync.dma_start(out=whh0T, in_=t_whh0T.ap().rearrange("p (k g m) -> p k g m", k=KT_H, g=NGATE))
            wih1T = const.tile([128, KT_H, NGATE, SL], FR)
            nc.sync.dma_start(out=wih1T, in_=t_wih1T.ap().rearrange("p (k g m) -> p k g m", k=KT_H, g=NGATE))
            whh1T = const.tile([128, KT_H, NGATE, SL], FR)
            nc.sync.dma_start(out=whh1T, in_=t_whh1T.ap().rearrange("p (k g m) -> p k g m", k=KT_H, g=NGATE))
            wcT = const.tile([128, KT_H, NGATE, SL], FR)
            nc.sync.dma_start(out=wcT, in_=t_wcT.ap().rearrange("p (k g m) -> p k g m", k=KT_H, g=NGATE))
            fcw1T = const.tile([128, KT_H, H], FR)
            nc.sync.dma_start(out=fcw1T, in_=t_fcw1T.ap().rearrange("p (k m) -> p k m", k=KT_H))
            fcw2T = const.tile([128, KT_H, YD], FR)
            nc.sync.dma_start(out=fcw2T, in_=t_fcw2T.ap().rearrange("p (k m) -> p k m", k=KT_H))
            b0 = const.tile([128, NGATE], F32)
            nc.sync.dma_start(out=b0, in_=t_b0.ap())
            b0d = const.tile([128, NGATE], F32)
            nc.sync.dma_start(out=b0d, in_=t_b0d.ap())
            b1 = const.tile([128, NGATE], F32)
            nc.sync.dma_start(out=b1, in_=t_b1.ap())
            fcb1 = const.tile([128, KT_H], F32)
            nc.sync.dma_start(out=fcb1, in_=t_fcb1.ap())
            fcb2 = const.tile([YD, 1], F32)
            nc.sync.dma_start(out=fcb2, in_=t_fcb2.ap())

            # ---- persistent state ----
            est = const.tile([YD, B], F32)  # replicated running estimate
            nc.sync.dma_start(out=est, in_=t_lastyT.ap())
            est_r = const.tile([YD, B], FR)
            nc.vector.tensor_copy(est_r, est)
            c0 = const.tile([128, B], F32)
            nc.vector.memset(c0, 0.0)
            c1 = const.tile([128, B], F32)
            nc.vector.memset(c1, 0.0)

            def allgather_h(hk, tagsuffix):
                """Exchange one [128,B] hidden slice: bounce -> AG -> 2-half
                gather.  Returns (half_a, half_b): [128, KH, B] SBUF tiles."""
                inb = dbounce.tile([128, B], FR, tag="agi" + tagsuffix, name="agi")
                nc.sync.dma_start(out=inb, in_=hk[:])
                outb = dshared.tile([NCORES * 128, B], FR, tag="ago" + tagsuffix,
                                    name="ago", addr_space="Shared")
                nc.gpsimd.collective_compute(
                    "AllGather", mybir.AluOpType.bypass, replica_groups=RG,
                    ins=[inb[:].opt()], outs=[outb[:].opt()],
                )
                ha = hfull.tile([128, KH, B], FR, tag="ha" + tagsuffix, name="ha")
                hb = hfull.tile([128, KH, B], FR, tag="hb" + tagsuffix, name="hb")
                src = outb[:].rearrange("(k p) b -> p k b", p=128)
                nc.sync.dma_start(out=ha[:, :, :], in_=src[:, 0:KH, :])
                nc.sync.dma_start(out=hb[:, :, :], in_=src[:, KH:KT_H, :])
                return ha, hb

            def mm_half(z, wT, hhalf, g, off, start, stop):
                """One gather-half (KH k-tiles) of a full-hidden contraction."""
                for k in range(KH):
                    nc.tensor.matmul(z, wT[:, off + k, g, :], hhalf[:, k, :],
                                     start=(start and k == 0),
                                     stop=(stop and k == KH - 1))

            def mm_hid_split(z, wT, ha, hb, start, stop):
                """All gates' a-half matmuls, then all gates' b-halves, so the
                first half runs while the second gather DMA is landing."""
                for g in range(NGATE):
                    mm_half(z[g], wT, ha, g, 0, start, False)
                for g in range(NGATE):
                    mm_half(z[g], wT, hb, g, KH, False, stop)

            def lstm_halfstep(zp, bias, cprev, tagp):
                """Gate activations + cell update. zp: 4 PSUM tiles [128,B]."""
                gi = gact.tile([128, B], F32, tag="gi", name="gi")
                gf = gact.tile([128, B], F32, tag="gf", name="gf")
                gg = gact.tile([128, B], F32, tag="gg", name="gg")
                go = gact.tile([128, B], F32, tag="go", name="go")
                nc.scalar.activation(gi, zp[0], AF.Sigmoid, bias=bias[:, 0:1])
                nc.scalar.activation(gf, zp[1], AF.Sigmoid, bias=bias[:, 1:2])
                nc.scalar.activation(gg, zp[2], AF.Tanh, bias=bias[:, 2:3])
                nc.scalar.activation(go, zp[3], AF.Sigmoid, bias=bias[:, 3:4])
                fc_ = gact.tile([128, B], F32, tag="fc_", name="fc_")
                nc.vector.tensor_mul(fc_, gf, cprev)
                ig = gact.tile([128, B], F32, tag="ig", name="ig")
                nc.vector.tensor_mul(ig, gi, gg)
                cnew = state.tile([128, B], F32, tag=tagp, name="cnew")
                nc.vector.tensor_add(cnew, fc_, ig)
                tc_ = gact.tile([128, B], F32, tag="tc_", name="tc_")
                nc.scalar.activation(tc_, cnew, AF.Tanh)
                hnew = state.tile([128, B], FR, tag=tagp + "h", name="hnew")
                nc.vector.tensor_mul(hnew, go, tc_)
                return cnew, hnew

            # ================= encode =================
            h0fa = h0fb = None  # gathered h0_{s-1}
            h1fa = h1fb = None  # gathered h1_{s-2}
            xt = xload.tile([128, 2, B], FR, tag="x", name="xt")
            nc.sync.dma_start(out=xt, in_=t_xpre.ap()[0])
            for s in range(PRE_LEN):
                # ---- L0_s: x-part free; hh0 right when h0f_{s-1} lands ----
                z0 = [psum.tile([128, B], F32, tag="z", name=f"z0g{g}") for g in range(NGATE)]
                for g in range(NGATE):
                    nc.tensor.matmul(z0[g], w0xT[:, 0, g, :], xt[:, 0, :],
                                     start=True, stop=False)
                    nc.tensor.matmul(z0[g], w0xT[:, 1, g, :], xt[:, 1, :],
                                     start=False, stop=(s == 0))
                if s + 1 < PRE_LEN:
                    xt = xload.tile([128, 2, B], FR, tag="x", name="xt")
                    nc.sync.dma_start(out=xt, in_=t_xpre.ap()[s + 1])
                if s > 0:
                    mm_hid_split(z0, whh0T, h0fa, h0fb, start=False, stop=True)
                c0, h0k = lstm_halfstep(z0, b0, c0, "c0")
                h0fa_n, h0fb_n = allgather_h(h0k, "0")

                # ---- L1_{s-1}: wih1 (h0f_{s-1}) first, whh1 (h1f_{s-2}) LAST
                # so the h1 gather may land a full period late.
                if s >= 1:
                    z1 = [psum.tile([128, B], F32, tag="z", name=f"z1g{g}") for g in range(NGATE)]
                    mm_hid_split(z1, wih1T, h0fa, h0fb, start=True, stop=(s == 1))
                    if s >= 2:
                        mm_hid_split(z1, whh1T, h1fa, h1fb, start=False, stop=True)
                    c1, h1k = lstm_halfstep(z1, b1, c1, "c1")
                    h1fa, h1fb = allgather_h(h1k, "1")
                h0fa, h0fb = h0fa_n, h0fb_n

            # ---- encode flush: L1_63 (h1_63 = "top" for the decode FC) ----
            z1 = [psum.tile([128, B], F32, tag="z", name=f"zfg{g}") for g in range(NGATE)]
            mm_hid_split(z1, wih1T, h0fa, h0fb, start=True, stop=False)
            mm_hid_split(z1, whh1T, h1fa, h1fb, start=False, stop=True)
            c1, h1k = lstm_halfstep(z1, b1, c1, "c1")
            h1fa, h1fb = allgather_h(h1k, "1")

            # ================= decode =================
            xt = xload.tile([128, 2, B], FR, tag="x", name="xt")
            nc.sync.dma_start(out=xt[:, 0, :], in_=t_xfwd.ap()[0, 0:128, :])
            nc.sync.dma_start(out=xt[0:64, 1, :], in_=t_xfwd.ap()[0, 128:XD, :])
            for t in range(FWD_LEN):
                last = t == FWD_LEN - 1
                # ---- L0_t pre-runs (free while AG1_{t-1} is in flight):
                # x-part, hh0 on h0f_{t-1}, West @ est_{t-1}.
                if not last:
                    z0 = [psum.tile([128, B], F32, tag="z", name=f"z0g{g}") for g in range(NGATE)]
                    for g in range(NGATE):
                        nc.tensor.matmul(z0[g], w0xT[:, 0, g, :], xt[:, 0, :],
                                         start=True, stop=False)
                        nc.tensor.matmul(z0[g], w0xT[0:64, 1, g, :], xt[0:64, 1, :],
                                         start=False, stop=False)
                        nc.tensor.matmul(z0[g], westT[:, g, :], est_r,
                                         start=False, stop=False)
                    mm_hid_split(z0, whh0T, h0fa, h0fb, start=False, stop=False)
                    if t + 1 < FWD_LEN:
                        xt = xload.tile([128, 2, B], FR, tag="x", name="xt")
                        nc.sync.dma_start(out=xt[:, 0, :], in_=t_xfwd.ap()[t + 1, 0:128, :])
                        nc.sync.dma_start(out=xt[0:64, 1, :], in_=t_xfwd.ap()[t + 1, 128:XD, :])

                # ---- FC head: u = tanh(fc_w1 h1f + b); gates close off Wc@u;
                # est recursion (fc_w2 u) runs off the critical path.
                u = gact.tile([128, KT_H, B], FR, tag="u", name="u")
                pp = psum.tile([YD, B], F32, tag="z", name="pp")
                for m in range(KT_H):
                    up = psum.tile([128, B], F32, tag="z", name="up")
                    for k in range(KH):
                        nc.tensor.matmul(up, fcw1T[:, k, m * 128:(m + 1) * 128],
                                         h1fa[:, k, :], start=(k == 0), stop=False)
                    for k in range(KH):
                        nc.tensor.matmul(up, fcw1T[:, KH + k, m * 128:(m + 1) * 128],
                                         h1fb[:, k, :], start=False, stop=(k == KH - 1))
                    nc.scalar.activation(u[:, m, :], up, AF.Tanh,
                                         bias=fcb1[:, m:m + 1])
                    if not last:
                        for g in range(NGATE):
                            nc.tensor.matmul(z0[g], wcT[:, m, g, :], u[:, m, :],
                                             start=False,
                                             stop=(m == KT_H - 1))
                    nc.tensor.matmul(pp, fcw2T[:, m, :], u[:, m, :],
                                     start=(m == 0), stop=(m == KT_H - 1))
                if not last:
                    c0, h0k = lstm_halfstep(z0, b0d, c0, "c0")
                    h0fa_n, h0fb_n = allgather_h(h0k, "0")
                # est update, output, f16 copy for next step's West pre-run
                estn = state.tile([YD, B], F32, tag="est", name="estn")
                nc.vector.tensor_add(estn, est, pp)
                nc.vector.tensor_scalar_add(estn, estn, fcb2[:, 0:1])
                est = estn
                nc.sync.dma_start(out=t_out.ap()[t], in_=est)
                if last:
                    break
                est_r = state.tile([YD, B], FR, tag="estr", name="est_r")
                nc.vector.tensor_copy(est_r, est)

                # ---- L1_t: whh1 (h1f_{t-1}, landed) first; wih1 (h0f_t) last
                z1 = [psum.tile([128, B], F32, tag="z", name=f"z1g{g}") for g in range(NGATE)]
                mm_hid_split(z1, whh1T, h1fa, h1fb, start=True, stop=False)
                mm_hid_split(z1, wih1T, h0fa_n, h0fb_n, start=False, stop=True)
                c1, h1k = lstm_halfstep(z1, b1, c1, "c1")
                h1fa, h1fb = allgather_h(h1k, "1")
                h0fa, h0fb = h0fa_n, h0fb_n

    nc.compile()
    return nc


def kernel(**inputs) -> np.ndarray:
    from concourse.bass_utils import run_bass_kernel_spmd

    key = "prog"
    if key not in _CACHE:
        _CACHE[key] = _build_program()
    nc = _CACHE[key]

    in_maps = _shard_host(inputs)
    res = run_bass_kernel_spmd(nc, in_maps, core_ids=list(range(NCORES)))
    est = np.asarray(res.results[0]["est_out"])  # (FWD, YD, B)
    return est.transpose(0, 2, 1).astype(np.float32).copy()  # (FWD, B, YD)


# revision 13
# speedup vs baseline: 1.3430x; 1.3430x over previous
"""Trainium2 Bass kernel for nn_DiffNet (2-layer LSTM encoder/decoder + FC head).

Sharding: tensor-parallel over the hidden/gate dimension across 8 NeuronCores.
Core k owns hidden rows [k*128, (k+1)*128) of both LSTM layers plus the
matching rows of fc_w1 / columns of fc_w2.  Activations are [hidden, batch] so
the full batch (256) is the matmul moving dimension.

v3 pipeline:
- Two AllGathers per step (h0, h1), each issued the moment its payload is
  ready; the other layer's matmuls run during the in-flight collective.
- Emission order keeps both recurrence loops at period ~= AG-chain + tail:
  encode: [x_s | hh0_s -> AG0_s | wih1(L1_{s-1}) | whh1(L1_{s-1}) -> AG1].
  whh1 is the LAST accumulation of L1 so its operand (the h1 gather) can land
  one full period late.
- Gathers land in two k-halves; path-critical contractions emit all gates'
  first-half matmuls before any second-half ones, so half the tail hides
  under the gather DMA.
- Decode folds fc2 into the layer-0 gate weights (Wc = W_est @ fc_w2), so
  the gates close straight off the tanh layer; the est recursion
  (est_t = est_{t-1} + fc_w2 u_t + fc_b2) runs off the critical path.
  W_est@fc_b2 is folded into the layer-0 bias on the host.

Self-contained: hardcodes all shapes; host-side numpy only reshapes/slices.
"""

import os

import numpy as np

L = 2
H = 1024
XD = 192
YD = 64
IN = XD + YD  # 256
B = 256
PRE_LEN = int(os.environ.get("DIFFNET_PRE", "64"))
FWD_LEN = int(os.environ.get("DIFFNET_FWD", "48"))
NCORES = 8
SL = H // NCORES  # 128 hidden rows per core
KT_H = H // 128  # 8 K-tiles to contract over a full hidden vector
KH = KT_H // 2  # k-tiles per gather half
NGATE = 4

_CACHE = {}


def _shard_host(inputs):
    """Build per-core input dicts (numpy only: slice / transpose / reshape)."""
    f32 = np.float32

    pre_x = np.asarray(inputs["pre_x"], f32)
    pre_y = np.asarray(inputs["pre_y"], f32)
    fwd_x = np.asarray(inputs["forward_x"], f32)

    # Encoder input, step-major, [t, p(128), kt(2), b] so the DMA is contiguous
    xy = np.concatenate([pre_x, pre_y], axis=2)  # (PRE, B, IN)
    xpre = (
        xy.transpose(0, 2, 1)  # (PRE, IN, B)
        .reshape(PRE_LEN, 2, 128, B)
        .transpose(0, 2, 1, 3)  # (PRE, 128, 2, B)
        .astype(np.float16)
    )
    # Decoder exogenous input: [t, in(192), b]
    xfwd = fwd_x.transpose(0, 2, 1).astype(np.float16)  # (FWD, 192, B)

    w_ih_0 = np.asarray(inputs["w_ih_0"], f32).reshape(NGATE, H, IN)
    w_hh_0 = np.asarray(inputs["w_hh_0"], f32).reshape(NGATE, H, H)
    w_ih_1 = np.asarray(inputs["w_ih_1"], f32).reshape(NGATE, H, H)
    w_hh_1 = np.asarray(inputs["w_hh_1"], f32).reshape(NGATE, H, H)
    b0 = (np.asarray(inputs["b_ih_0"], f32) + np.asarray(inputs["b_hh_0"], f32)).reshape(NGATE, H)
    b1 = (np.asarray(inputs["b_ih_1"], f32) + np.asarray(inputs["b_hh_1"], f32)).reshape(NGATE, H)
    fc_w1 = np.asarray(inputs["fc_w1"], f32)
    fc_b1 = np.asarray(inputs["fc_b1"], f32)
    fc_w2 = np.asarray(inputs["fc_w2"], f32)
    fc_b2 = np.asarray(inputs["fc_b2"], f32)

    def lhsT_tiles(sl_w):
        """(4, 128, K) gate-major rows-for-this-core -> lhsT [128, KT*4*128]."""
        kdim = sl_w.shape[2]
        kt = kdim // 128
        return (
            sl_w.transpose(2, 0, 1)  # (K, 4, 128)
            .reshape(kt, 128, NGATE, SL)
            .transpose(1, 0, 2, 3)  # (128, kt, 4, 128)
            .reshape(128, kt * NGATE * SL)
            .copy()
        )

    maps = []
    for k in range(NCORES):
        sl = slice(k * SL, (k + 1) * SL)
        w0xT = lhsT_tiles(w_ih_0[:, sl, :])  # (128, 2*4*128)
        west = w_ih_0[:, sl, XD:]  # (4, 128, 64)
        westT = west.transpose(2, 0, 1).reshape(YD, NGATE * SL).copy()  # (64, 512)
        whh0T = lhsT_tiles(w_hh_0[:, sl, :])  # (128, 8*4*128)
        wih1T = lhsT_tiles(w_ih_1[:, sl, :])
        whh1T = lhsT_tiles(w_hh_1[:, sl, :])
        # fc2 folded into the layer-0 gates: Wc[g,r,:] = west[g,r,:] @ fc_w2
        wc = np.einsum("grc,ch->grh", west.astype(np.float64),
                       fc_w2.astype(np.float64)).astype(f32)  # (4, 128, H)
        wcT = lhsT_tiles(wc)  # (128, 8*4*128)
        # ... and W_est @ fc_b2 into the layer-0 bias (decode only!)
        b0d = b0[:, sl] + west.astype(np.float64) @ fc_b2.astype(np.float64)  # (4,128)
        # FC head replicated on every core (small): no collective for est.
        fcw1T = (
            fc_w1.T.reshape(KT_H, 128, H).transpose(1, 0, 2).reshape(128, KT_H * H).copy()
        )
        fcw2T = (
            fc_w2.T.reshape(KT_H, 128, YD).transpose(1, 0, 2).reshape(128, KT_H * YD).copy()
        )
        m = {
            "xpre": xpre,
            "xfwd": xfwd,
            "w0xT": w0xT.astype(np.float16),
            "westT": westT.astype(np.float16),
            "whh0T": whh0T.astype(np.float16),
            "wih1T": wih1T.astype(np.float16),
            "whh1T": whh1T.astype(np.float16),
            "wcT": wcT.astype(np.float16),
            "fcw1T": fcw1T.astype(np.float16),
            "fcw2T": fcw2T.astype(np.float16),
            "b0": b0[:, sl].T.copy(),  # (128, 4)
            "b0d": np.asarray(b0d, f32).T.copy(),  # (128, 4) decode bias
            "b1": b1[:, sl].T.copy(),
            "fcb1": fc_b1.reshape(KT_H, 128).T.copy(),  # (128, 8): bias per M-tile
            "fcb2": fc_b2.reshape(YD, 1).copy(),
            "lastyT": pre_y[-1].T.copy(),  # (64, 256)
        }
        maps.append(m)
    return maps


def _build_program():
    import concourse.bass as bass
    import concourse.mybir as mybir
    import concourse.tile as tile
    from concourse import bacc

    dt = mybir.dt
    AF = mybir.ActivationFunctionType
    F32 = dt.float32
    FR = dt.float16  # matmul operand dtype (FWL stays on, ~8x bf16 precision)

    nc = bacc.Bacc("TRN2", target_bir_lowering=False, debug=False, num_devices=NCORES)

    # ---- external I/O ----
    t_xpre = nc.dram_tensor("xpre", [PRE_LEN, 128, 2, B], FR, kind="ExternalInput")
    t_xfwd = nc.dram_tensor("xfwd", [FWD_LEN, XD, B], FR, kind="ExternalInput")
    t_w0xT = nc.dram_tensor("w0xT", [128, 2 * NGATE * SL], FR, kind="ExternalInput")
    t_westT = nc.dram_tensor("westT", [YD, NGATE * SL], FR, kind="ExternalInput")
    t_whh0T = nc.dram_tensor("whh0T", [128, KT_H * NGATE * SL], FR, kind="ExternalInput")
    t_wih1T = nc.dram_tensor("wih1T", [128, KT_H * NGATE * SL], FR, kind="ExternalInput")
    t_whh1T = nc.dram_tensor("whh1T", [128, KT_H * NGATE * SL], FR, kind="ExternalInput")
    t_wcT = nc.dram_tensor("wcT", [128, KT_H * NGATE * SL], FR, kind="ExternalInput")
    t_fcw1T = nc.dram_tensor("fcw1T", [128, KT_H * H], FR, kind="ExternalInput")
    t_fcw2T = nc.dram_tensor("fcw2T", [128, KT_H * YD], FR, kind="ExternalInput")
    t_b0 = nc.dram_tensor("b0", [128, NGATE], F32, kind="ExternalInput")
    t_b0d = nc.dram_tensor("b0d", [128, NGATE], F32, kind="ExternalInput")
    t_b1 = nc.dram_tensor("b1", [128, NGATE], F32, kind="ExternalInput")
    t_fcb1 = nc.dram_tensor("fcb1", [128, KT_H], F32, kind="ExternalInput")
    t_fcb2 = nc.dram_tensor("fcb2", [YD, 1], F32, kind="ExternalInput")
    t_lastyT = nc.dram_tensor("lastyT", [YD, B], F32, kind="ExternalInput")
    t_out = nc.dram_tensor("est_out", [FWD_LEN, YD, B], F32, kind="ExternalOutput")

    RG = [list(range(NCORES))]

    with tile.TileContext(nc) as tc:
        with (
            tc.tile_pool(name="const", bufs=1) as const,
            tc.tile_pool(name="xload", bufs=3) as xload,
            tc.tile_pool(name="state", bufs=4) as state,
            tc.tile_pool(name="gact", bufs=6) as gact,
            tc.tile_pool(name="hfull", bufs=3) as hfull,
            tc.tile_pool(name="psum", bufs=8, space="PSUM") as psum,
            tc.tile_pool(name="dbounce", bufs=8, space="DRAM") as dbounce,
            tc.tile_pool(name="dshared", bufs=8, space="DRAM") as dshared,
        ):
            # ---- load constants ----
            w0xT = const.tile([128, 2, NGATE, SL], FR)
            nc.sync.dma_start(out=w0xT, in_=t_w0xT.ap().rearrange("p (k g m) -> p k g m", k=2, g=NGATE))
            westT = const.tile([YD, NGATE, SL], FR)
            nc.sync.dma_start(out=westT, in_=t_westT.ap().rearrange("p (g m) -> p g m", g=NGATE))
            whh0T = const.tile([128, KT_H, NGATE, SL], FR)
            nc.sync.dma_start(out=whh0T, in_=t_whh0T.ap().rearrange("p (k g m) -> p k g m", k=KT_H, g=NGATE))
            wih1T = const.tile([128, KT_H, NGATE, SL], FR)
            nc.sync.dma_start(out=wih1T, in_=t_wih1T.ap().rearrange("p (k g m) -> p k g m", k=KT_H, g=NGATE))
            whh1T = const.tile([128, KT_H, NGATE, SL], FR)
            nc.sync.dma_start(out=whh1T, in_=t_whh1T.ap().rearrange("p (k g m) -> p k g m", k=KT_H, g=NGATE))
            wcT = const.tile([128, KT_H, NGATE, SL], FR)
            nc.sync.dma_start(out=wcT, in_=t_wcT.ap().rearrange("p (k g m) -> p k g m", k=KT_H, g=NGATE))
            fcw1T = const.tile([128, KT_H, H], FR)
            nc.sync.dma_start(out=fcw1T, in_=t_fcw1T.ap().rearrange("p (k m) -> p k m", k=KT_H))
            fcw2T = const.tile([128, KT_H, YD], FR)
            nc.sync.dma_start(out=fcw2T, in_=t_fcw2T.ap().rearrange("p (k m) -> p k m", k=KT_H))
            b0 = const.tile([128, NGATE], F32)
            nc.sync.dma_start(out=b0, in_=t_b0.ap())
            b0d = const.tile([128, NGATE], F32)
            nc.sync.dma_start(out=b0d, in_=t_b0d.ap())
            b1 = const.tile([128, NGATE], F32)
            nc.sync.dma_start(out=b1, in_=t_b1.ap())
            fcb1 = const.tile([128, KT_H], F32)
            nc.sync.dma_start(out=fcb1, in_=t_fcb1.ap())
            fcb2 = const.tile([YD, 1], F32)
            nc.sync.dma_start(out=fcb2, in_=t_fcb2.ap())

            # ---- persistent state ----
            est = const.tile([YD, B], F32)  # replicated running estimate
            nc.sync.dma_start(out=est, in_=t_lastyT.ap())
            est_r = const.tile([YD, B], FR)
            nc.vector.tensor_copy(est_r, est)
            c0 = const.tile([128, B], F32)
            nc.vector.memset(c0, 0.0)
            c1 = const.tile([128, B], F32)
            nc.vector.memset(c1, 0.0)

            def allgather_h(hk, tagsuffix):
                """Exchange one [128,B] hidden slice: bounce -> AG -> 2-half
                gather.  Returns (half_a, half_b): [128, KH, B] SBUF tiles."""
                inb = dbounce.tile([128, B], FR, tag="agi" + tagsuffix, name="agi")
                nc.sync.dma_start(out=inb, in_=hk[:])
                outb = dshared.tile([NCORES * 128, B], FR, tag="ago" + tagsuffix,
                                    name="ago", addr_space="Shared")
                nc.gpsimd.collective_compute(
                    "AllGather", mybir.AluOpType.bypass, replica_groups=RG,
                    ins=[inb[:].opt()], outs=[outb[:].opt()],
                )
                ha = hfull.tile([128, KH, B], FR, tag="ha" + tagsuffix, name="ha")
                hb = hfull.tile([128, KH, B], FR, tag="hb" + tagsuffix, name="hb")
                src = outb[:].rearrange("(k p) b -> p k b", p=128)
                nc.sync.dma_start(out=ha[:, :, :], in_=src[:, 0:KH, :])
                nc.sync.dma_start(out=hb[:, :, :], in_=src[:, KH:KT_H, :])
                return ha, hb

            def mm_half(z, wT, hhalf, g, off, start, stop):
                """One gather-half (KH k-tiles) of a full-hidden contraction."""
                for k in range(KH):
                    nc.tensor.matmul(z, wT[:, off + k, g, :], hhalf[:, k, :],
                                     start=(start and k == 0),
                                     stop=(stop and k == KH - 1))

            def mm_hid_split(z, wT, ha, hb, start, stop):
                """All gates' a-half matmuls, then all gates' b-halves, so the
                first half runs while the second gather DMA is landing."""
                for g in range(NGATE):
                    mm_half(z[g], wT, ha, g, 0, start, False)
                for g in range(NGATE):
                    mm_half(z[g], wT, hb, g, KH, False, stop)

            def lstm_halfstep(zp, bias, cprev, tagp):
                """Gate activations + cell update. zp: 4 PSUM tiles [128,B]."""
                gi = gact.tile([128, B], F32, tag="gi", name="gi")
                gf = gact.tile([128, B], F32, tag="gf", name="gf")
                gg = gact.tile([128, B], F32, tag="gg", name="gg")
                go = gact.tile([128, B], F32, tag="go", name="go")
                nc.scalar.activation(gi, zp[0], AF.Sigmoid, bias=bias[:, 0:1])
                nc.scalar.activation(gf, zp[1], AF.Sigmoid, bias=bias[:, 1:2])
                nc.scalar.activation(gg, zp[2], AF.Tanh, bias=bias[:, 2:3])
                nc.scalar.activation(go, zp[3], AF.Sigmoid, bias=bias[:, 3:4])
                fc_ = gact.tile([128, B], F32, tag="fc_", name="fc_")
                nc.vector.tensor_mul(fc_, gf, cprev)
                ig = gact.tile([128, B], F32, tag="ig", name="ig")
                nc.vector.tensor_mul(ig, gi, gg)
                cnew = state.tile([128, B], F32, tag=tagp, name="cnew")
                nc.vector.tensor_add(cnew, fc_, ig)
                tc_ = gact.tile([128, B], F32, tag="tc_", name="tc_")
                nc.scalar.activation(tc_, cnew, AF.Tanh)
                hnew = state.tile([128, B], FR, tag=tagp + "h", name="hnew")
                nc.vector.tensor_mul(hnew, go, tc_)
                return cnew, hnew

            # ================= encode =================
            h0fa = h0fb = None  # gathered h0_{s-1}
            h1fa = h1fb = None  # gathered h1_{s-2}
            xt = xload.tile([128, 2, B], FR, tag="x", name="xt")
            nc.sync.dma_start(out=xt, in_=t_xpre.ap()[0])
            for s in range(PRE_LEN):
                # ---- L0_s: x-part free; hh0 right when h0f_{s-1} lands ----
                z0 = [psum.tile([128, B], F32, tag="z", name=f"z0g{g}") for g in range(NGATE)]
                for g in range(NGATE):
                    nc.tensor.matmul(z0[g], w0xT[:, 0, g, :], xt[:, 0, :],
                                     start=True, stop=False)
                    nc.tensor.matmul(z0[g], w0xT[:, 1, g, :], xt[:, 1, :],
                                     start=False, stop=(s == 0))
                if s + 1 < PRE_LEN:
                    xt = xload.tile([128, 2, B], FR, tag="x", name="xt")
                    nc.sync.dma_start(out=xt, in_=t_xpre.ap()[s + 1])
                if s > 0:
                    mm_hid_split(z0, whh0T, h0fa, h0fb, start=False, stop=True)
                c0, h0k = lstm_halfstep(z0, b0, c0, "c0")
                h0fa_n, h0fb_n = allgather_h(h0k, "0")

                # ---- L1_{s-1}: wih1 (h0f_{s-1}) first, whh1 (h1f_{s-2}) LAST
                # so the h1 gather may land a full period late.
                if s >= 1:
                    z1 = [psum.tile([128, B], F32, tag="z", name=f"z1g{g}") for g in range(NGATE)]
                    mm_hid_split(z1, wih1T, h0fa, h0fb, start=True, stop=(s == 1))
                    if s >= 2:
                        mm_hid_split(z1, whh1T, h1fa, h1fb, start=False, stop=True)
                    c1, h1k = lstm_halfstep(z1, b1, c1, "c1")
                    h1fa, h1fb = allgather_h(h1k, "1")
                h0fa, h0fb = h0fa_n, h0fb_n

            # ---- encode flush: L1_63 (h1_63 = "top" for the decode FC) ----
            z1 = [psum.tile([128, B], F32, tag="z", name=f"zfg{g}") for g in range(NGATE)]
            mm_hid_split(z1, wih1T, h0fa, h0fb, start=True, stop=False)
            mm_hid_split(z1, whh1T, h1fa, h1fb, start=False, stop=True)
            c1, h1k = lstm_halfstep(z1, b1, c1, "c1")
            h1fa, h1fb = allgather_h(h1k, "1")

            # ================= decode =================
            xt = xload.tile([128, 2, B], FR, tag="x", name="xt")
            nc.sync.dma_start(out=xt[:, 0, :], in_=t_xfwd.ap()[0, 0:128, :])
            nc.sync.dma_start(out=xt[0:64, 1, :], in_=t_xfwd.ap()[0, 128:XD, :])
            for t in range(FWD_LEN):
                last = t == FWD_LEN - 1
                # ---- L0_t pre-runs (free while AG1_{t-1} is in flight):
                # x-part, hh0 on h0f_{t-1}, West @ est_{t-1}.
                if not last:
                    z0 = [psum.tile([128, B], F32, tag="z", name=f"z0g{g}") for g in range(NGATE)]
                    for g in range(NGATE):
                        nc.tensor.matmul(z0[g], w0xT[:, 0, g, :], xt[:, 0, :],
                                         start=True, stop=False)
                        nc.tensor.matmul(z0[g], w0xT[0:64, 1, g, :], xt[0:64, 1, :],
                                         start=False, stop=False)
                        nc.tensor.matmul(z0[g], westT[:, g, :], est_r,
                                         start=False, stop=False)
                    mm_hid_split(z0, whh0T, h0fa, h0fb, start=False, stop=False)
                    if t + 1 < FWD_LEN:
                        xt = xload.tile([128, 2, B], FR, tag="x", name="xt")
                        nc.sync.dma_start(out=xt[:, 0, :], in_=t_xfwd.ap()[t + 1, 0:128, :])
                        nc.sync.dma_start(out=xt[0:64, 1, :], in_=t_xfwd.ap()[t + 1, 128:XD, :])

                # ---- FC head: u = tanh(fc_w1 h1f + b); gates close off Wc@u;
                # est recursion (fc_w2 u) runs off the critical path.
                u = gact.tile([128, KT_H, B], FR, tag="u", name="u")
                pp = psum.tile([YD, B], F32, tag="z", name="pp")
                for m in range(KT_H):
                    up = psum.tile([128, B], F32, tag="z", name="up")
                    for k in range(KH):
                        nc.tensor.matmul(up, fcw1T[:, k, m * 128:(m + 1) * 128],
                                         h1fa[:, k, :], start=(k == 0), stop=False)
                    for k in range(KH):
                        nc.tensor.matmul(up, fcw1T[:, KH + k, m * 128:(m + 1) * 128],
                                         h1fb[:, k, :], start=False, stop=(k == KH - 1))
                    nc.scalar.activation(u[:, m, :], up, AF.Tanh,
                                         bias=fcb1[:, m:m + 1])
                    if not last:
                        for g in range(NGATE):
                            nc.tensor.matmul(z0[g], wcT[:, m, g, :], u[:, m, :],
                                             start=False,
                                             stop=(m == KT_H - 1))
                    nc.tensor.matmul(pp, fcw2T[:, m, :], u[:, m, :],
                                     start=(m == 0), stop=(m == KT_H - 1))
                if not last:
                    c0, h0k = lstm_halfstep(z0, b0d, c0, "c0")
                    h0fa_n, h0fb_n = allgather_h(h0k, "0")
                # est update, output, f16 copy for next step's West pre-run
                estn = state.tile([YD, B], F32, tag="est", name="estn")
                nc.vector.tensor_add(estn, est, pp)
                nc.vector.tensor_scalar_add(estn, estn, fcb2[:, 0:1])
                est = estn
                nc.sync.dma_start(out=t_out.ap()[t], in_=est)
                if last:
                    break
                est_r = state.tile([YD, B], FR, tag="estr", name="est_r")
                nc.vector.tensor_copy(est_r, est)

                # ---- L1_t: whh1 (h1f_{t-1}, landed) first; wih1 (h0f_t) last
                z1 = [psum.tile([128, B], F32, tag="z", name=f"z1g{g}") for g in range(NGATE)]
                mm_hid_split(z1, whh1T, h1fa, h1fb, start=True, stop=False)
                mm_hid_split(z1, wih1T, h0fa_n, h0fb_n, start=False, stop=True)
                c1, h1k = lstm_halfstep(z1, b1, c1, "c1")
                h1fa, h1fb = allgather_h(h1k, "1")
                h0fa, h0fb = h0fa_n, h0fb_n

    nc.compile()
    return nc


def kernel(**inputs) -> np.ndarray:
    from concourse.bass_utils import run_bass_kernel_spmd

    key = "prog"
    if key not in _CACHE:
        _CACHE[key] = _build_program()
    nc = _CACHE[key]

    in_maps = _shard_host(inputs)
    res = run_bass_kernel_spmd(nc, in_maps, core_ids=list(range(NCORES)))
    est = np.asarray(res.results[0]["est_out"])  # (FWD, YD, B)
    return est.transpose(0, 2, 1).astype(np.float32).copy()  # (FWD, B, YD)
